# revision 53
# baseline (speedup 1.0000x reference)
"""LocallyConnected2d (3x3, 64x64 out, C_in=16, C_out=32, B=32) on 8 trn2 cores.

out[b,o,h,w] = sum_{c,i,j} x[b,c,h+i,w+j] * weight[0,o,c,h,w,(i,j)] + bias[0,o,h,w]

Sharding: spatial over H_out - core i computes output rows 8i..8i+8, needing
input rows 8i..8i+10 (halo) and its 1/8 slice of the (per-position, unique)
weights.

Per position (h,w): K=145 contraction (9 taps x 16 ch + ones row carrying the
bias), M=32 (C_out). Split K=128 (taps 0-7, host-replicated shifted-window
bands) + 17 (tap 8 + ones, its own host-shifted band), the K=17 part
PSUM-accumulated onto the K=128 part.

Quad packing: 4 adjacent positions share one matmul - lhsT [K, 4x32] and
rhs [K, 4x32] produce a [128, 128] PSUM block whose 32x32 diagonal blocks are
the 4 positions' [C_out, B] outputs. PSUM accumulation groups are per 2KB
zero region (= bank = 4 quads): matmul start=True zeroes the whole bank, so
each bank runs one group of 8 matmuls (4 A + 4 B).

DMA structure (the perf-critical part): HWDGE slot recycling couples DMA N
with DMA N-8 in emission order, and per-DMA throughput scales with packet
(per-partition contiguous run) size. So: few DMAs (11 input DMAs total),
partition-major DRAM layouts giving 4-33KB packets, emitted in completion
order with the row-0 working set in the first slots. Outputs ride the
separate SWDGE (gpsimd) slot pool as per-row-pair DMAs from a compact f32
stage filled by 4 diagonal-extract DVE copies per row.
"""

import numpy as np

import concourse.bass as bass
import concourse.mybir as mybir
import concourse.tile as tile
from concourse import bacc
from concourse import bass_utils

N_CORES = 8
B, CI, CO = 32, 16, 32
H = W = 64
HL = H // N_CORES          # output rows per core
XROWS = HL + 2             # input rows per core (with halo)
XW = 66
XFLAT = XROWS * XW         # 660
T = HL * XW                # 528
NQ = W // 4                # 16 quads per row
KA, KB = 128, 17

_cache = {}


def _np_bf16():
    import ml_dtypes
    return np.dtype(ml_dtypes.bfloat16)


def _build(parts="both"):
    dt = mybir.dt.bfloat16
    f32 = mybir.dt.float32
    nc = bacc.Bacc("TRN2", target_bir_lowering=False, debug=False,
                   num_devices=N_CORES)
    # partition-major DRAM + row-PAIR slices: 8-16KB per-partition runs,
    # which the DGE turns into big packets (4KB packets pay ~600ns each)
    xr_d = nc.dram_tensor("xr", [KA, T, B], dt, kind="ExternalInput")
    wa_d = nc.dram_tensor("wa", [KA, HL, W, CO], dt, kind="ExternalInput")
    wb_d = nc.dram_tensor("wb", [CI, HL, W, CO], dt, kind="ExternalInput")
    bias_d = nc.dram_tensor("biasx", [KA, HL, NQ], mybir.dt.float32,
                            kind="ExternalInput")
    out_d = nc.dram_tensor("out", [KA, HL, NQ, B], f32,
                           kind="ExternalOutput")

    with tile.TileContext(nc) as tc:
        with (
            tc.tile_pool(name="ppa", bufs=1) as ppa,
            tc.tile_pool(name="pw", bufs=1) as pw,
            tc.tile_pool(name="pwb", bufs=1) as pwb,
            tc.tile_pool(name="pbi", bufs=1) as pbi,
            tc.tile_pool(name="pst", bufs=4) as pst,
            tc.tile_pool(name="pp", bufs=4, space=bass.MemorySpace.PSUM) as pp,
        ):
            pa = ppa.tile([KA, T, B], dt, tag="pa")
            wa = pw.tile([KA, HL, W, CO], dt, tag="wa")
            wb = pwb.tile([CI, HL, W, CO], dt, tag="wb")
            bi = pbi.tile([KA, HL, NQ], mybir.dt.float32, tag="bi")

            # Emission order = HWDGE slot order; first slots hold the
            # row-0/1 working set, later DMAs recycle slots of DMAs that
            # finish first. Partition-major DRAM + multi-row slices =>
            # 8KB contiguous packets (small packets pay ~600ns fixed).
            # wb is 16-partition (DMAs to <128 partitions don't fan out),
            # so it goes as 8 per-row DMAs round-robin'd across engines on
            # the gpsimd SWDGE queue; xr row 1 rides the same queue so the
            # critical rows 0-1 load three-queues-wide.
            nc.scalar.dma_start(wa[:, 0, 0:36], wa_d[:, 0, 0:36])
            nc.sync.dma_start(pa[:, 0:36], xr_d[:, 0:36])
            nc.gpsimd.dma_start(wb[:, 0], wb_d[:, 0])
            nc.scalar.dma_start(pa[:, XW:XW + 36], xr_d[:, XW:XW + 36])
            nc.sync.dma_start(wa[:, 1, 0:36], wa_d[:, 1, 0:36])
            nc.gpsimd.dma_start(wb[:, 1], wb_d[:, 1])
            nc.scalar.dma_start(wa[:, 0, 36:64], wa_d[:, 0, 36:64])
            nc.sync.dma_start(pa[:, 36:XW], xr_d[:, 36:XW])
            nc.scalar.dma_start(pa[:, XW + 36:2 * XW],
                                xr_d[:, XW + 36:2 * XW])
            nc.sync.dma_start(wa[:, 1, 36:64], wa_d[:, 1, 36:64])
            nc.gpsimd.dma_start(bi[:], bias_d[:])
            for h in range(2, HL):
                nc.gpsimd.dma_start(wb[:, h], wb_d[:, h])
            # rows 2-7 in pairs: weights on scalar, x on sync
            for g in range(1, HL // 2):
                sl = slice(2 * XW * g, 2 * XW * (g + 1))
                hs = slice(2 * g, 2 * g + 2)
                nc.scalar.dma_start(wa[:, hs], wa_d[:, hs])
                nc.sync.dma_start(pa[:, sl], xr_d[:, sl])

            NH = NQ // 2
            for h in range(HL):
                st = pst.tile([KA, NQ, B], f32, tag="st")
                t0r = XW * h
                # half-row PSUM tiles (2 banks each, 4 in flight): finer
                # psum release keeps the PE pipeline from draining between
                # rows (it never leaves the fast p-state)
                for half in range(2):
                    ps = pp.tile([KA, NH, KA], f32, tag="ps")
                    qs = slice(NH * half, NH * (half + 1))
                    mma = [(ps[:, q, :], wa[:, h, 4 * qq:4 * qq + 4, :],
                            pa[:, t0r + 4 * qq:t0r + 4 * qq + 4, :])
                           for q, qq in enumerate(range(NH * half,
                                                        NH * (half + 1)))]
                    # tap-8 band == the shift-133 band (stored first,
                    # partitions 0:16 - matmul rhs base partition must be
                    # 0/32/64) + 1 col
                    mmb = [(ps[:, q, :], wb[:, h, 4 * qq:4 * qq + 4, :],
                            pa[0:16, t0r + 4 * qq + 1:t0r + 4 * qq + 5, :])
                           for q, qq in enumerate(range(NH * half,
                                                        NH * (half + 1)))]
                    if parts == "both":
                        for g in range(NH // 4):
                            for d in range(4):
                                o, l, r = mma[4 * g + d]
                                nc.tensor.matmul(o, l, r, start=(d == 0),
                                                 stop=False)
                            for d in range(4):
                                o, l, r = mmb[4 * g + d]
                                nc.tensor.matmul(o, l, r, start=False,
                                                 stop=(d == 3))
                    else:
                        mm = mma if parts == "a" else mmb
                        for q in range(NH):
                            o, l, r = mm[q]
                            nc.tensor.matmul(o, l, r, start=True, stop=True)

                    for j in range(4):
                        sl = slice(32 * j, 32 * (j + 1))
                        nc.vector.tensor_add(
                            st[sl, qs, :], ps[sl, :, sl],
                            bi[sl, h].unsqueeze(2)[:, qs]
                            .to_broadcast([32, NH, B]))

                # outputs split across the (by now idle) HWDGE queues
                nc.sync.dma_start(out_d[0:64, h], st[0:64])
                nc.scalar.dma_start(out_d[64:128, h], st[64:128])
    nc.compile()
    return nc


def _get_nc(parts="both"):
    if parts not in _cache:
        _cache[parts] = _build(parts)
    return _cache[parts]


def _pack_inputs(x, weight, bias):
    """Full inputs -> per-core in_maps (host-side shard + relayout)."""
    bf16 = _np_bf16()
    x = np.asarray(x, np.float32)
    weight = np.asarray(weight, np.float32)
    bias = np.asarray(bias, np.float32)

    # weights: [1,o,c,h,w,k] -> [h, w, k=(tap,kc), o]
    wt = weight[0].transpose(2, 3, 4, 1, 0).reshape(H, W, 9 * CI, CO)
    # bias -> [128=(j,o), h, q] f32: out position w = 4q + j
    bias_t = bias[0].reshape(CO, H, NQ, 4).transpose(3, 0, 1, 2)  # [j,o,h,q]
    bias_t = bias_t.reshape(KA, H, NQ)

    in_maps = []
    for c in range(N_CORES):
        r0 = HL * c
        xsl = x[:, :, r0:r0 + XROWS, :]                     # [b, ci, 10, 66]
        xs = xsl.transpose(1, 2, 3, 0).reshape(CI, XFLAT, B)
        xr = np.zeros((KA, T, B), np.float32)
        # band slot order: tap 7 (shift 133) first so the B-part (tap 8 =
        # shift 134) can read slot 0 at +1 col with base partition 0
        perm = [7, 0, 1, 2, 3, 4, 5, 6]
        for s, k in enumerate(perm):
            i, j = divmod(k, 3)
            off = XW * i + j
            wk = min(T, XFLAT - off)
            xr[16 * s:16 * (s + 1), :wk, :] = xs[:, off:off + wk, :]

        wc = wt[r0:r0 + HL].transpose(2, 0, 1, 3)           # [k, h, w, o]
        wca = np.concatenate([wc[16 * k:16 * (k + 1)] for k in perm])
        in_maps.append({
            "xr": np.ascontiguousarray(xr, dtype=bf16),
            "wa": np.ascontiguousarray(wca, dtype=bf16),
            "wb": np.ascontiguousarray(wc[KA:KA + CI], dtype=bf16),
            "biasx": np.ascontiguousarray(bias_t[:, r0:r0 + HL, :],
                                         dtype=np.float32),
        })
    return in_maps


def _gather(results):
    # per-core out: [128=(j,o), HL, NQ, B]; w = 4q + j
    outs = np.stack([results[c]["out"] for c in range(N_CORES)])
    o6 = outs.reshape(N_CORES, 4, CO, HL, NQ, B)   # [core, j, o, h, q, b]
    out = o6.transpose(5, 2, 0, 3, 4, 1)           # [b, o, core, h, q, j]
    return np.ascontiguousarray(out.reshape(B, CO, H, W))


def run(x, weight, bias, parts="both", **spmd_kwargs):
    nc = _get_nc(parts)
    in_maps = _pack_inputs(x, weight, bias)
    res = bass_utils.run_bass_kernel_spmd(nc, in_maps,
                                          core_ids=list(range(N_CORES)),
                                          **spmd_kwargs)
    return _gather(res.results), res


def kernel(x, weight, bias):
    out, _ = run(x, weight, bias)
    return out


# revision 56
# speedup vs baseline: 1.0591x; 1.0591x over previous
"""LocallyConnected2d (3x3, 64x64 out, C_in=16, C_out=32, B=32) on 8 trn2 cores.

out[b,o,h,w] = sum_{c,i,j} x[b,c,h+i,w+j] * weight[0,o,c,h,w,(i,j)] + bias[0,o,h,w]

Sharding: spatial over H_out - core i computes output rows 8i..8i+8, needing
input rows 8i..8i+10 (halo) and its 1/8 slice of the (per-position, unique)
weights.

Per position (h,w): K=145 contraction (9 taps x 16 ch + ones row carrying the
bias), M=32 (C_out). Split K=128 (taps 0-7, host-replicated shifted-window
bands) + 17 (tap 8 + ones, its own host-shifted band), the K=17 part
PSUM-accumulated onto the K=128 part.

Quad packing: 4 adjacent positions share one matmul - lhsT [K, 4x32] and
rhs [K, 4x32] produce a [128, 128] PSUM block whose 32x32 diagonal blocks are
the 4 positions' [C_out, B] outputs. PSUM accumulation groups are per 2KB
zero region (= bank = 4 quads): matmul start=True zeroes the whole bank, so
each bank runs one group of 8 matmuls (4 A + 4 B).

DMA structure (the perf-critical part): HWDGE slot recycling couples DMA N
with DMA N-8 in emission order, and per-DMA throughput scales with packet
(per-partition contiguous run) size. So: few DMAs (11 input DMAs total),
partition-major DRAM layouts giving 4-33KB packets, emitted in completion
order with the row-0 working set in the first slots. Outputs ride the
separate SWDGE (gpsimd) slot pool as per-row-pair DMAs from a compact f32
stage filled by 4 diagonal-extract DVE copies per row.
"""

import numpy as np

import concourse.bass as bass
import concourse.mybir as mybir
import concourse.tile as tile
from concourse import bacc
from concourse import bass_utils

N_CORES = 8
B, CI, CO = 32, 16, 32
H = W = 64
HL = H // N_CORES          # output rows per core
XROWS = HL + 2             # input rows per core (with halo)
XW = 66
XFLAT = XROWS * XW         # 660
T = HL * XW                # 528
NQ = W // 4                # 16 quads per row
KA, KB = 128, 17

_cache = {}


def _np_bf16():
    import ml_dtypes
    return np.dtype(ml_dtypes.bfloat16)


def _build(parts="both"):
    dt = mybir.dt.bfloat16
    f32 = mybir.dt.float32
    nc = bacc.Bacc("TRN2", target_bir_lowering=False, debug=False,
                   num_devices=N_CORES)
    # partition-major DRAM + row-PAIR slices: 8-16KB per-partition runs,
    # which the DGE turns into big packets (4KB packets pay ~600ns each)
    xr_d = nc.dram_tensor("xr", [KA, T, B], dt, kind="ExternalInput")
    wa_d = nc.dram_tensor("wa", [KA, HL, W, CO], dt, kind="ExternalInput")
    wb_d = nc.dram_tensor("wb", [CI, HL, W, CO], dt, kind="ExternalInput")
    bias_d = nc.dram_tensor("biasx", [KA, HL, NQ], mybir.dt.float32,
                            kind="ExternalInput")
    out_d = nc.dram_tensor("out", [KA, HL, NQ, B], f32,
                           kind="ExternalOutput")

    with tile.TileContext(nc) as tc:
        with (
            tc.tile_pool(name="ppa", bufs=1) as ppa,
            tc.tile_pool(name="pw", bufs=1) as pw,
            tc.tile_pool(name="pwb", bufs=1) as pwb,
            tc.tile_pool(name="pbi", bufs=1) as pbi,
            tc.tile_pool(name="pst", bufs=4) as pst,
            tc.tile_pool(name="pp", bufs=4, space=bass.MemorySpace.PSUM) as pp,
        ):
            pa = ppa.tile([KA, T, B], dt, tag="pa")
            wa = pw.tile([KA, HL, W, CO], dt, tag="wa")
            wb = pwb.tile([CI, HL, W, CO], dt, tag="wb")
            bi = pbi.tile([KA, HL, NQ], mybir.dt.float32, tag="bi")

            # Emission order = HWDGE slot order; first slots hold the
            # row-0/1 working set, later DMAs recycle slots of DMAs that
            # finish first. Partition-major DRAM + multi-row slices =>
            # 8KB contiguous packets (small packets pay ~600ns fixed).
            # wb is 16-partition (DMAs to <128 partitions don't fan out),
            # so it goes as 8 per-row DMAs round-robin'd across engines on
            # the gpsimd SWDGE queue; xr row 1 rides the same queue so the
            # critical rows 0-1 load three-queues-wide.
            nc.scalar.dma_start(wa[:, 0, 0:36], wa_d[:, 0, 0:36])
            nc.sync.dma_start(pa[:, 0:36], xr_d[:, 0:36])
            nc.gpsimd.dma_start(wb[:, 0], wb_d[:, 0])
            nc.scalar.dma_start(pa[:, XW:XW + 36], xr_d[:, XW:XW + 36])
            nc.sync.dma_start(wa[:, 1, 0:36], wa_d[:, 1, 0:36])
            nc.gpsimd.dma_start(wb[:, 1], wb_d[:, 1])
            nc.scalar.dma_start(wa[:, 0, 36:64], wa_d[:, 0, 36:64])
            nc.sync.dma_start(pa[:, 36:XW], xr_d[:, 36:XW])
            nc.scalar.dma_start(pa[:, XW + 36:2 * XW],
                                xr_d[:, XW + 36:2 * XW])
            nc.sync.dma_start(wa[:, 1, 36:64], wa_d[:, 1, 36:64])
            nc.gpsimd.dma_start(bi[:], bias_d[:])
            for h in range(2, HL):
                nc.gpsimd.dma_start(wb[:, h], wb_d[:, h])
            # rows 2-3 as singles (arrive sooner), 4-7 in pairs (bigger
            # packets); weights on scalar, x on sync
            for h in range(2, 4):
                nc.scalar.dma_start(wa[:, h], wa_d[:, h])
                nc.sync.dma_start(pa[:, XW * h:XW * (h + 1)],
                                  xr_d[:, XW * h:XW * (h + 1)])
            for g in range(2, HL // 2):
                sl = slice(2 * XW * g, 2 * XW * (g + 1))
                hs = slice(2 * g, 2 * g + 2)
                nc.scalar.dma_start(wa[:, hs], wa_d[:, hs])
                nc.sync.dma_start(pa[:, sl], xr_d[:, sl])

            NH = NQ // 2
            for h in range(HL):
                st = pst.tile([KA, NQ, B], f32, tag="st")
                t0r = XW * h
                # half-row PSUM tiles (2 banks each, 4 in flight): finer
                # psum release keeps the PE pipeline from draining between
                # rows (it never leaves the fast p-state)
                for half in range(2):
                    ps = pp.tile([KA, NH, KA], f32, tag="ps")
                    qs = slice(NH * half, NH * (half + 1))
                    mma = [(ps[:, q, :], wa[:, h, 4 * qq:4 * qq + 4, :],
                            pa[:, t0r + 4 * qq:t0r + 4 * qq + 4, :])
                           for q, qq in enumerate(range(NH * half,
                                                        NH * (half + 1)))]
                    # tap-8 band == the shift-133 band (stored first,
                    # partitions 0:16 - matmul rhs base partition must be
                    # 0/32/64) + 1 col
                    mmb = [(ps[:, q, :], wb[:, h, 4 * qq:4 * qq + 4, :],
                            pa[0:16, t0r + 4 * qq + 1:t0r + 4 * qq + 5, :])
                           for q, qq in enumerate(range(NH * half,
                                                        NH * (half + 1)))]
                    if parts == "both":
                        for g in range(NH // 4):
                            for d in range(4):
                                o, l, r = mma[4 * g + d]
                                nc.tensor.matmul(o, l, r, start=(d == 0),
                                                 stop=False)
                            for d in range(4):
                                o, l, r = mmb[4 * g + d]
                                nc.tensor.matmul(o, l, r, start=False,
                                                 stop=(d == 3))
                    else:
                        mm = mma if parts == "a" else mmb
                        for q in range(NH):
                            o, l, r = mm[q]
                            nc.tensor.matmul(o, l, r, start=True, stop=True)

                    for j in range(4):
                        sl = slice(32 * j, 32 * (j + 1))
                        nc.vector.tensor_add(
                            st[sl, qs, :], ps[sl, :, sl],
                            bi[sl, h].unsqueeze(2)[:, qs]
                            .to_broadcast([32, NH, B]))

                # outputs split across the (by now idle) HWDGE queues
                nc.sync.dma_start(out_d[0:64, h], st[0:64])
                nc.scalar.dma_start(out_d[64:128, h], st[64:128])
    nc.compile()
    return nc


def _get_nc(parts="both"):
    if parts not in _cache:
        _cache[parts] = _build(parts)
    return _cache[parts]


def _pack_inputs(x, weight, bias):
    """Full inputs -> per-core in_maps (host-side shard + relayout)."""
    bf16 = _np_bf16()
    x = np.asarray(x, np.float32)
    weight = np.asarray(weight, np.float32)
    bias = np.asarray(bias, np.float32)

    # weights: [1,o,c,h,w,k] -> [h, w, k=(tap,kc), o]
    wt = weight[0].transpose(2, 3, 4, 1, 0).reshape(H, W, 9 * CI, CO)
    # bias -> [128=(j,o), h, q] f32: out position w = 4q + j
    bias_t = bias[0].reshape(CO, H, NQ, 4).transpose(3, 0, 1, 2)  # [j,o,h,q]
    bias_t = bias_t.reshape(KA, H, NQ)

    in_maps = []
    for c in range(N_CORES):
        r0 = HL * c
        xsl = x[:, :, r0:r0 + XROWS, :]                     # [b, ci, 10, 66]
        xs = xsl.transpose(1, 2, 3, 0).reshape(CI, XFLAT, B)
        xr = np.zeros((KA, T, B), np.float32)
        # band slot order: tap 7 (shift 133) first so the B-part (tap 8 =
        # shift 134) can read slot 0 at +1 col with base partition 0
        perm = [7, 0, 1, 2, 3, 4, 5, 6]
        for s, k in enumerate(perm):
            i, j = divmod(k, 3)
            off = XW * i + j
            wk = min(T, XFLAT - off)
            xr[16 * s:16 * (s + 1), :wk, :] = xs[:, off:off + wk, :]

        wc = wt[r0:r0 + HL].transpose(2, 0, 1, 3)           # [k, h, w, o]
        wca = np.concatenate([wc[16 * k:16 * (k + 1)] for k in perm])
        in_maps.append({
            "xr": np.ascontiguousarray(xr, dtype=bf16),
            "wa": np.ascontiguousarray(wca, dtype=bf16),
            "wb": np.ascontiguousarray(wc[KA:KA + CI], dtype=bf16),
            "biasx": np.ascontiguousarray(bias_t[:, r0:r0 + HL, :],
                                         dtype=np.float32),
        })
    return in_maps


def _gather(results):
    # per-core out: [128=(j,o), HL, NQ, B]; w = 4q + j
    outs = np.stack([results[c]["out"] for c in range(N_CORES)])
    o6 = outs.reshape(N_CORES, 4, CO, HL, NQ, B)   # [core, j, o, h, q, b]
    out = o6.transpose(5, 2, 0, 3, 4, 1)           # [b, o, core, h, q, j]
    return np.ascontiguousarray(out.reshape(B, CO, H, W))


def run(x, weight, bias, parts="both", **spmd_kwargs):
    nc = _get_nc(parts)
    in_maps = _pack_inputs(x, weight, bias)
    res = bass_utils.run_bass_kernel_spmd(nc, in_maps,
                                          core_ids=list(range(N_CORES)),
                                          **spmd_kwargs)
    return _gather(res.results), res


def kernel(x, weight, bias):
    out, _ = run(x, weight, bias)
    return out
